# revision 1
# baseline (speedup 1.0000x reference)
"""Masked spatial RMSE loss on 8 trn2 NeuronCores.

reference math:
    sq      = (y - yhat)^2                      [B, N]
    spatial = sq @ W.T                          [B, N]   (W = spots_neighbors)
    loss    = sqrt(sum((sq + spatial) * m) / sum(m) + eps)

We never materialize spatial: with the trace identity
    sum(m * (sq @ W.T)) = sum(W * (m.T @ sq))
the big contraction becomes C = m.T @ sq (contraction over batch),
immediately reduced against W.  Sharding: columns n of sq/W are split
across the 8 cores (each core reads its [B, N/8] slice of yhat/y, its
[N, N/8] slice of W, and the full mask as the matmul's stationary
operand) - 24MB of HBM per core instead of the 73MB a data-parallel
split would need.  Each core emits per-partition partials of
S2 = sum(W os C), S1 = sum(m os sq), cnt = sum(m); the host combines and
takes the sqrt.
"""

import numpy as np

B = 2048
N = 4096
NCORES = 8
NS = N // NCORES  # 512 columns per core
P = 128  # partitions
T = B // P  # 16 batch tiles
IC = N // P  # 32 i-chunks (rows of C per 128)
G = 4  # i-chunk groups (8 psum banks each)
ICG = IC // G  # 8 chunks per group
GW = ICG * P  # 1024 group column width
EPS = 1e-6

_CACHE: dict = {}


def build_program(repeat=1, parts=("sq", "s1", "mm", "drain"), mmdt="bf16"):
    import concourse.bass as bass  # noqa: F401
    import concourse.tile as tile
    from concourse import bacc, mybir

    f32 = mybir.dt.float32
    bf16 = mybir.dt.bfloat16
    u8 = mybir.dt.uint8
    f8 = mybir.dt.float8e4
    mdt = bf16 if mmdt == "bf16" else f8
    Alu = mybir.AluOpType
    Act = mybir.ActivationFunctionType

    nc = bacc.Bacc(
        "TRN2", target_bir_lowering=False, debug=False, num_devices=NCORES
    )

    yhat_d = nc.dram_tensor("yhat_s", [B, NS], f32, kind="ExternalInput").ap()
    y_d = nc.dram_tensor("y_s", [B, NS], f32, kind="ExternalInput").ap()
    mask_d = nc.dram_tensor("mask", [B, N], u8, kind="ExternalInput").ap()
    masks_d = nc.dram_tensor("mask_s", [B, NS], u8, kind="ExternalInput").ap()
    w_d = nc.dram_tensor("w_s", [N, NS], f32, kind="ExternalInput").ap()
    out_d = nc.dram_tensor("out", [P, 4], f32, kind="ExternalOutput").ap()

    mask_v = mask_d.rearrange("(t p) i -> p t i", p=P)  # [128, 16, 4096]
    masks_v = masks_d.rearrange("(t p) n -> p t n", p=P)  # [128, 16, 512]
    yhat_v = yhat_d.rearrange("(t p) n -> p t n", p=P)
    y_v = y_d.rearrange("(t p) n -> p t n", p=P)
    w_v = w_d.rearrange("(c p) n -> p c n", p=P)  # [128, 32, 512]

    H = 2 * G  # mask halves: 1MB DMA granules, 4 i-chunks each
    HW_ = GW // 2  # 512 columns per half

    with tile.TileContext(nc) as tc:
        with (
            tc.tile_pool(name="persist", bufs=1) as persist,
            tc.tile_pool(name="mstage", bufs=4) as mstage,
            tc.tile_pool(name="mbf", bufs=4) as mbfp,
            tc.tile_pool(name="wg", bufs=2) as wgp,
            tc.tile_pool(name="io", bufs=2) as iop,
            tc.tile_pool(name="scratch", bufs=2) as scratch,
            tc.tile_pool(name="psum", bufs=8, space="PSUM") as psum,
        ):
            for rep in range(repeat):
                sq_bf = persist.tile([P, T, NS], bf16)
                sq_mm = sq_bf
                if mmdt == "fp8dr":
                    sq_f8 = persist.tile([P, T, NS], f8)
                    sq_mm = sq_f8
                ms_bf = persist.tile([P, T, NS], bf16)
                acc = persist.tile([P, IC], f32)
                s1c = persist.tile([P, T], f32)
                cntc = persist.tile([P, G], f32)
                if "drain" not in parts:
                    nc.vector.memset(acc, 0.0)
                if "s1" not in parts:
                    nc.vector.memset(s1c, 0.0)
                    nc.vector.memset(cntc, 0.0)
                if "sq" not in parts:
                    nc.vector.memset(sq_bf, 0.0)
                    nc.vector.memset(ms_bf, 0.0)

                # ---- DMA emission order is the schedule: the sync HWDGE
                # ring is FIFO, so interleave sq pieces / mask halves /
                # W quarters so the PE is never input-starved.
                mh = [None] * H      # mask half u8 tiles
                mbh = [None] * H     # mask half bf16 tiles
                wq = [None] * G      # W per-phase tiles
                yhp = [None] * (T // 2)
                yyp = [None] * (T // 2)

                def dma_sq_piece(j):
                    yhp[j] = iop.tile([P, 2, NS], f32, tag="yh", name=f"yh{rep}_{j}")
                    yyp[j] = iop.tile([P, 2, NS], f32, tag="yy", name=f"yy{rep}_{j}")
                    sl = slice(2 * j, 2 * j + 2)
                    nc.sync.dma_start(out=yhp[j], in_=yhat_v[:, sl, :])
                    nc.sync.dma_start(out=yyp[j], in_=y_v[:, sl, :])

                def dma_mh(h):
                    mh[h] = mstage.tile(
                        [P, T, HW_], u8, tag="mh", name=f"mh{rep}_{h}", bufs=3
                    )
                    nc.sync.dma_start(
                        out=mh[h], in_=mask_v[:, :, h * HW_ : (h + 1) * HW_]
                    )

                def dma_wq(g):
                    wq[g] = wgp.tile(
                        [P, ICG, NS], f32, tag="wt", name=f"wq{rep}_{g}"
                    )
                    nc.sync.dma_start(
                        out=wq[g], in_=w_v[:, g * ICG : (g + 1) * ICG, :]
                    )

                def dma_ms():
                    t_ = mstage.tile([P, T, NS], u8, tag="msu", name=f"msu{rep}", bufs=1)
                    nc.sync.dma_start(out=t_, in_=masks_v)
                    return t_

                # front-loaded interleave (1MB granules, ~3us each)
                dma_sq_piece(0)
                dma_mh(0)
                dma_sq_piece(1)
                dma_mh(1)
                dma_sq_piece(2)
                dma_sq_piece(3)
                dma_wq(0)
                dma_sq_piece(4)
                dma_sq_piece(5)
                dma_mh(2)
                dma_sq_piece(6)
                dma_sq_piece(7)
                dma_mh(3)
                dma_wq(1)
                ms_u8 = dma_ms()
                dma_mh(4)
                dma_mh(5)
                dma_wq(2)
                dma_mh(6)
                dma_mh(7)
                dma_wq(3)

                # ---- sq compute (piece-wise, behind the DMAs)
                if "sq" in parts:
                    for j in range(T // 2):
                        sl = slice(2 * j, 2 * j + 2)
                        d_t = scratch.tile([P, 2, NS], f32, tag="d")
                        nc.vector.tensor_sub(d_t, yyp[j], yhp[j])
                        nc.scalar.activation(sq_bf[:, sl, :], d_t, Act.Square)
                        if mmdt == "fp8dr":
                            nc.gpsimd.tensor_copy(
                                out=sq_f8[:, sl, :], in_=sq_bf[:, sl, :]
                            )

                if "s1" in parts:
                    nc.gpsimd.tensor_copy(out=ms_bf, in_=ms_u8)

                # ---- mask half casts (u8 -> bf16), alternate ACT/Pool
                def cast_mbh(h):
                    mbh[h] = mbfp.tile(
                        [P, T, HW_], mdt, tag="mb", name=f"mbh{rep}_{h}"
                    )
                    for j in range(T // 2):
                        sl = slice(2 * j, 2 * j + 2)
                        if j % 2 == 0:
                            nc.scalar.copy(out=mbh[h][:, sl, :], in_=mh[h][:, sl, :])
                        else:
                            nc.gpsimd.tensor_copy(
                                out=mbh[h][:, sl, :], in_=mh[h][:, sl, :]
                            )

                # ---- main contraction, 4 phases x 8 chunks (8 psum banks)
                if "mm" in parts:
                    cast_mbh(0)
                    cast_mbh(1)
                    for g in range(G):
                        if 2 * g + 2 < H:
                            cast_mbh(2 * g + 2)
                        if 2 * g + 3 < H:
                            cast_mbh(2 * g + 3)
                        ps_list = [
                            psum.tile([P, NS], f32, tag="ps", name=f"ps{rep}_{g}_{k}")
                            for k in range(ICG)
                        ]
                        if mmdt == "fp8dr":
                            import concourse.mybir as _mb
                            DR = _mb.MatmulPerfMode.DoubleRow
                            TP = T // 2
                            def mm_dr(k, tp):
                                src = mbh[2 * g + k // 4]
                                cs = slice((k % 4) * P, (k % 4 + 1) * P)
                                ts_ = slice(2 * tp, 2 * tp + 2)
                                nc.tensor.matmul(
                                    ps_list[k],
                                    lhsT=src[:, ts_, cs],
                                    rhs=sq_mm[:, ts_, :],
                                    start=(tp == 0),
                                    stop=(tp == TP - 1),
                                    perf_mode=DR,
                                )
                            if g == 0:
                                for tp in range(TP):
                                    for k in range(ICG):
                                        mm_dr(k, tp)
                            else:
                                for k in range(ICG):
                                    for tp in range(TP):
                                        mm_dr(k, tp)
                        elif g == 0:
                            # sq pieces stream in t-order: t-major
                            for t in range(T):
                                for k in range(ICG):
                                    src = mbh[2 * g + k // 4]
                                    nc.tensor.matmul(
                                        ps_list[k],
                                        lhsT=src[:, t, (k % 4) * P : (k % 4 + 1) * P],
                                        rhs=sq_mm[:, t, :],
                                        start=(t == 0),
                                        stop=(t == T - 1),
                                    )
                        else:
                            # sq resident: chunk-major so drains overlap MMs
                            for k in range(ICG):
                                src = mbh[2 * g + k // 4]
                                for t in range(T):
                                    nc.tensor.matmul(
                                        ps_list[k],
                                        lhsT=src[:, t, (k % 4) * P : (k % 4 + 1) * P],
                                        rhs=sq_mm[:, t, :],
                                        start=(t == 0),
                                        stop=(t == T - 1),
                                    )
                        if "drain" in parts:
                            for k in range(ICG):
                                tr2 = scratch.tile([P, NS], f32, tag="tr2")
                                nc.vector.scalar_tensor_tensor(
                                    out=tr2,
                                    in0=ps_list[k],
                                    scalar=1.0,
                                    in1=wq[g][:, k, :],
                                    op0=Alu.mult,
                                    op1=Alu.mult,
                                    accum_out=acc[:, g * ICG + k : g * ICG + k + 1],
                                )
                        if "s1" in parts:
                            for t in range(4 * g, 4 * g + 4):
                                tr1 = scratch.tile([P, NS], f32, tag="tr1")
                                nc.vector.scalar_tensor_tensor(
                                    out=tr1,
                                    in0=sq_bf[:, t, :],
                                    scalar=1.0,
                                    in1=ms_bf[:, t, :],
                                    op0=Alu.mult,
                                    op1=Alu.mult,
                                    accum_out=s1c[:, t : t + 1],
                                )
                            trc = scratch.tile([P, 4, NS], bf16, tag="trc")
                            nc.scalar.activation(
                                trc,
                                ms_bf[:, 4 * g : 4 * g + 4, :],
                                Act.Copy,
                                accum_out=cntc[:, g : g + 1],
                            )

                # pack partials: out[:, 0]=S2, out[:, 1]=S1, out[:, 2]=cnt
                out_sb = persist.tile([P, 4], f32)
                nc.vector.memset(out_sb, 0.0)
                nc.vector.tensor_reduce(
                    out=out_sb[:, 0:1], in_=acc, axis=mybir.AxisListType.X, op=Alu.add
                )
                nc.vector.tensor_reduce(
                    out=out_sb[:, 1:2], in_=s1c, axis=mybir.AxisListType.X, op=Alu.add
                )
                nc.vector.tensor_reduce(
                    out=out_sb[:, 2:3], in_=cntc, axis=mybir.AxisListType.X,
                    op=Alu.add
                )
                nc.sync.dma_start(out=out_d, in_=out_sb)

    nc.compile()
    return nc


def make_in_maps(yhat, y, batch_mask, spots_neighbors):
    mask_u8 = np.ascontiguousarray(batch_mask).view(np.uint8)
    yhat = np.ascontiguousarray(yhat, dtype=np.float32)
    y = np.ascontiguousarray(y, dtype=np.float32)
    w = np.ascontiguousarray(spots_neighbors, dtype=np.float32)
    in_maps = []
    for c in range(NCORES):
        sl = slice(c * NS, (c + 1) * NS)
        in_maps.append(
            {
                "yhat_s": np.ascontiguousarray(yhat[:, sl]),
                "y_s": np.ascontiguousarray(y[:, sl]),
                "mask": mask_u8,
                "mask_s": np.ascontiguousarray(mask_u8[:, sl]),
                "w_s": np.ascontiguousarray(w[:, sl]),
            }
        )
    return in_maps


def combine_outs(outs):
    s2 = 0.0
    s1 = 0.0
    cnt = 0.0
    for o in outs:
        o64 = o.astype(np.float64)
        s2 += o64[:, 0].sum()
        s1 += o64[:, 1].sum()
        cnt += o64[:, 2].sum()
    loss = np.sqrt((s1 + s2) / cnt + EPS)
    return np.array(loss, dtype=np.float32)


def kernel(yhat, y, batch_mask, spots_neighbors):
    from concourse.bass_utils import run_bass_kernel_spmd

    if "nc" not in _CACHE:
        _CACHE["nc"] = build_program()
    nc = _CACHE["nc"]
    in_maps = make_in_maps(yhat, y, batch_mask, spots_neighbors)
    res = run_bass_kernel_spmd(nc, in_maps, list(range(NCORES))).results
    return combine_outs([res[c]["out"] for c in range(NCORES)])



# revision 4
# speedup vs baseline: 4.6263x; 4.6263x over previous
"""Masked spatial RMSE loss on 8 trn2 NeuronCores.

reference math:
    sq      = (y - yhat)^2                      [B, N]
    spatial = sq @ W.T                          [B, N]   (W = spots_neighbors)
    loss    = sqrt(sum((sq + spatial) * m) / sum(m) + eps)

Identity used here: with W' = W + I,
    sum(m * (sq + sq @ W.T)) = sum(sq * (m @ W'))
so the one big contraction is V = m @ W' (contraction over i), immediately
reduced against sq.  The diagonal of W' folds the masked plain-sq term into
the same matmul, so no separate masked-sum pass is needed.

Sharding: columns n of W/y/yhat split across the 8 cores; the mask (the
contraction operand) is replicated, but shipped as 1-byte fp8 {0,1} so it
feeds the PE directly.  The matmul runs in fp8 DoubleRow mode (256-deep
contraction per instruction): the mask is exact in fp8, and W' is scaled by
1.25 on the host so {0,0.1,1.0,1.1} map to exactly representable fp8 values
{0,0.125,1.25,1.375}; the host divides the result by 1.25.  sq is computed
on-chip from bf16 inputs and stored fp8 for the final elementwise reduce
(vector STT with accum), where its rounding noise averages out across 8.4M
terms.

Per-core HBM traffic: mask 8MB + W' 2MB + y/yh 2+2MB = 14MB (vs 25MB for
the f32 layout), all streamed behind the PE.  The mask is host-packed into
per-b-tile [128 ipart, 32 ichunk, 128 b] blocks so every DMA is 4KB
contiguous per partition.  Each core's inputs are rolled along b by
-core*16 rows so the SPMD-identical count reduction (columns 0:16 of each
mask tile) counts a disjoint set of mask rows on every core.

Each core emits per-partition partials of S = sum(sq*(m@W'))*1.25 and
cnt = sum(m); the host combines, divides by 1.25, and takes the sqrt.
"""

import numpy as np

B = 2048
N = 4096
NCORES = 8
NS = N // NCORES  # 512 columns per core
P = 128  # partitions
T = B // P  # 16 batch tiles
IC = N // P  # 32 i-chunks (contraction)
GP = IC // 2  # 16 DoubleRow i-pair chunks
EPS = 1e-6
WSCALE = 1.25  # makes {0, 0.1, 1.0, 1.1} exact in fp8e4m3

_CACHE: dict = {}


def build_program(repeat=1, nwarm=10):
    import concourse.bass as bass  # noqa: F401
    import concourse.tile as tile
    from concourse import bacc, mybir

    f32 = mybir.dt.float32
    bf16 = mybir.dt.bfloat16
    f8 = mybir.dt.float8e4
    Alu = mybir.AluOpType
    Act = mybir.ActivationFunctionType
    DR = mybir.MatmulPerfMode.DoubleRow

    nc = bacc.Bacc(
        "TRN2", target_bir_lowering=False, debug=False, num_devices=NCORES
    )

    yhat_d = nc.dram_tensor("yhat_s", [B, NS], bf16, kind="ExternalInput").ap()
    y_d = nc.dram_tensor("y_s", [B, NS], bf16, kind="ExternalInput").ap()
    mt_d = nc.dram_tensor("mt", [B, N], f8, kind="ExternalInput").ap()
    wps_d = nc.dram_tensor("wps", [N, NS], f8, kind="ExternalInput").ap()
    out_d = nc.dram_tensor("out", [P, 4], f32, kind="ExternalOutput").ap()

    yhat_v = yhat_d.rearrange("(t p) n -> p t n", p=P)  # [128, 16, 512]
    y_v = y_d.rearrange("(t p) n -> p t n", p=P)
    # host pre-packed: mt[t*128 + p, c*128 + j] = mask[t*128 + j, c*128 + p]
    mt_v = mt_d.rearrange("(t p) (c j) -> p t c j", p=P, c=IC)
    wps_v = wps_d.rearrange("(c p) n -> p c n", p=P)  # [128, 32, 512]

    with tile.TileContext(nc) as tc:
        with (
            tc.tile_pool(name="persist", bufs=1) as persist,
            tc.tile_pool(name="mtp", bufs=16) as mtp,
            tc.tile_pool(name="io", bufs=3) as iop,
            tc.tile_pool(name="scratch", bufs=2) as scratch,
            tc.tile_pool(name="psum", bufs=8, space="PSUM") as psum,
        ):
            for rep in range(repeat):
                sq_f8 = persist.tile(
                    [P, T, NS], f8, tag="sq", bufs=2, name=f"sq{rep}"
                )
                wps_sb = persist.tile(
                    [P, IC, NS], f8, tag="wps", bufs=2, name=f"wps{rep}"
                )
                acc = persist.tile([P, T], f32, tag="acc", bufs=2, name=f"acc{rep}")
                cntc = persist.tile([P, T], f32, tag="cnt", bufs=2, name=f"cnt{rep}")

                # ---- warm the PE clock gate during the DMA lead-in
                # (first rep only: later reps in the R-variant stay warm)
                for w_ in range(nwarm if rep == 0 else 0):
                    if w_ == 0:
                        dum = persist.tile(
                            [P, 2, NS], f8, tag="dum", bufs=1, name=f"dum{rep}"
                        )
                        nc.vector.memset(dum, 0.0)
                    ps_w = psum.tile(
                        [P, NS], f32, tag="ps", name=f"psw{rep}_{w_}"
                    )
                    nc.tensor.matmul(
                        ps_w,
                        lhsT=dum[:, :, 0:P],
                        rhs=dum,
                        start=True,
                        stop=True,
                        perf_mode=DR,
                    )

                # ---- DMA emission order is the schedule (sync HWDGE ring
                # is FIFO): W' quarters + first mask tiles up front, then
                # mask tiles interleaved with y/yhat pieces.
                mts = [None] * T
                yhp = [None] * (T // 2)
                yyp = [None] * (T // 2)

                def dma_wq(q):
                    nc.sync.dma_start(
                        out=wps_sb[:, 8 * q : 8 * (q + 1), :],
                        in_=wps_v[:, 8 * q : 8 * (q + 1), :],
                    )

                def dma_mt(t):
                    mts[t] = mtp.tile(
                        [P, IC, P], f8, tag="mt", name=f"mt{rep}_{t}"
                    )
                    nc.sync.dma_start(out=mts[t], in_=mt_v[:, t])

                def dma_piece(j):
                    yhp[j] = iop.tile(
                        [P, 2, NS], bf16, tag="yh", name=f"yh{rep}_{j}"
                    )
                    yyp[j] = iop.tile(
                        [P, 2, NS], bf16, tag="yy", name=f"yy{rep}_{j}"
                    )
                    sl = slice(2 * j, 2 * j + 2)
                    nc.sync.dma_start(out=yhp[j], in_=yhat_v[:, sl, :])
                    nc.sync.dma_start(out=yyp[j], in_=y_v[:, sl, :])

                dma_wq(0)
                dma_mt(0)
                dma_wq(1)
                dma_mt(1)
                dma_wq(2)
                dma_wq(3)
                dma_piece(0)
                dma_mt(2)
                dma_piece(1)
                dma_mt(3)
                dma_piece(2)
                dma_mt(4)
                dma_piece(3)
                dma_mt(5)
                dma_mt(6)
                dma_piece(4)
                dma_mt(7)
                dma_mt(8)
                dma_piece(5)
                dma_mt(9)
                dma_mt(10)
                dma_piece(6)
                dma_mt(11)
                dma_mt(12)
                dma_piece(7)
                dma_mt(13)
                dma_mt(14)
                dma_mt(15)

                # ---- main loop: per b-tile, 16 DoubleRow MMs (contraction
                # over all 4096 i), then one STT drain against sq.
                for t in range(T):
                    if t % 2 == 0:
                        j = t // 2
                        d_t = scratch.tile(
                            [P, 2, NS], bf16, tag="d", name=f"d{rep}_{j}"
                        )
                        nc.vector.tensor_sub(d_t, yyp[j], yhp[j])
                        nc.scalar.activation(
                            sq_f8[:, 2 * j : 2 * j + 2, :], d_t, Act.Square
                        )
                    ps_t = psum.tile([P, NS], f32, tag="ps", name=f"ps{rep}_{t}")
                    for g in range(GP):
                        nc.tensor.matmul(
                            ps_t,
                            lhsT=mts[t][:, 2 * g : 2 * g + 2, :],
                            rhs=wps_sb[:, 2 * g : 2 * g + 2, :],
                            start=(g == 0),
                            stop=(g == GP - 1),
                            perf_mode=DR,
                        )
                    tr = scratch.tile([P, NS], f32, tag="tr", name=f"tr{rep}_{t}")
                    nc.vector.scalar_tensor_tensor(
                        out=tr,
                        in0=ps_t,
                        scalar=1.0,
                        in1=sq_f8[:, t, :],
                        op0=Alu.mult,
                        op1=Alu.mult,
                        accum_out=acc[:, t : t + 1],
                    )
                    # disjoint-per-core count slice (inputs are b-rolled)
                    ct = scratch.tile(
                        [P, IC, 16], bf16, tag="ct", name=f"ct{rep}_{t}"
                    )
                    nc.scalar.activation(
                        ct,
                        mts[t][:, :, 0:16],
                        Act.Copy,
                        accum_out=cntc[:, t : t + 1],
                    )

                # pack partials: out[:, 0]=S*1.25, out[:, 2]=cnt
                out_sb = persist.tile([P, 4], f32, tag="os", bufs=2, name=f"os{rep}")
                nc.vector.memset(out_sb, 0.0)
                nc.vector.tensor_reduce(
                    out=out_sb[:, 0:1], in_=acc, axis=mybir.AxisListType.X,
                    op=Alu.add
                )
                nc.vector.tensor_reduce(
                    out=out_sb[:, 2:3], in_=cntc, axis=mybir.AxisListType.X,
                    op=Alu.add
                )
                nc.sync.dma_start(out=out_d, in_=out_sb)

    nc.compile()
    return nc


def make_in_maps(yhat, y, batch_mask, spots_neighbors):
    import ml_dtypes

    f8 = ml_dtypes.float8_e4m3
    bf16 = ml_dtypes.bfloat16

    mask_u8 = np.ascontiguousarray(batch_mask).view(np.uint8)
    yhat = np.ascontiguousarray(yhat, dtype=np.float32)
    y = np.ascontiguousarray(y, dtype=np.float32)
    w = np.ascontiguousarray(spots_neighbors, dtype=np.float32)
    idx = np.arange(NS)

    in_maps = []
    for c in range(NCORES):
        sl = slice(c * NS, (c + 1) * NS)
        roll = -c * (P // NCORES)  # distinct count rows per core
        # wps = 1.25 * (W[:, sl] + I[:, sl]) in fp8 (all values exact for
        # W entries in {0, 0.1})
        wps = w[:, sl] * WSCALE
        wps[c * NS + idx, idx] += WSCALE
        wps8 = wps.astype(f8)
        # b-rolled inputs
        mrow = np.roll(mask_u8, roll, axis=0)
        yh = np.roll(yhat[:, sl], roll, axis=0).astype(bf16)
        yy = np.roll(y[:, sl], roll, axis=0).astype(bf16)
        # mask -> fp8 {0,1} bytes, packed transposed per b-tile:
        # mt[t*128 + p, c*128 + j] = mask[t*128 + j, c*128 + p]
        m8 = (mrow * 0x38).astype(np.uint8)
        mt = (
            m8.reshape(T, P, IC, P)
            .transpose(0, 3, 2, 1)
            .reshape(B, N)
        )
        mt = np.ascontiguousarray(mt).view(f8)
        in_maps.append(
            {
                "yhat_s": np.ascontiguousarray(yh),
                "y_s": np.ascontiguousarray(yy),
                "mt": mt,
                "wps": np.ascontiguousarray(wps8),
            }
        )
    return in_maps


def combine_outs(outs):
    s = 0.0
    cnt = 0.0
    for o in outs:
        o64 = o.astype(np.float64)
        s += o64[:, 0].sum()
        cnt += o64[:, 2].sum()
    loss = np.sqrt(s / WSCALE / cnt + EPS)
    return np.array(loss, dtype=np.float32)


def kernel(yhat, y, batch_mask, spots_neighbors):
    from concourse.bass_utils import run_bass_kernel_spmd

    if "nc" not in _CACHE:
        _CACHE["nc"] = build_program()
    nc = _CACHE["nc"]
    in_maps = make_in_maps(yhat, y, batch_mask, spots_neighbors)
    res = run_bass_kernel_spmd(nc, in_maps, list(range(NCORES))).results
    return combine_outs([res[c]["out"] for c in range(NCORES)])


# revision 5
# speedup vs baseline: 7.9334x; 1.7149x over previous
"""Masked spatial RMSE loss on 8 trn2 NeuronCores.

reference math:
    sq      = (y - yhat)^2                      [B, N]
    spatial = sq @ W.T                          [B, N]   (W = spots_neighbors)
    loss    = sqrt(sum((sq + spatial) * m) / sum(m) + eps)

Identity used here: with W' = W + I,
    sum(m * (sq + sq @ W.T)) = sum(sq * (m @ W'))
so the one big contraction is V = m @ W' (contraction over i), immediately
reduced against sq.  The diagonal of W' folds the masked plain-sq term into
the same matmul, so no separate masked-sum pass is needed.

Sharding: columns n of W/y/yhat split across the 8 cores; the mask (the
contraction operand) is replicated, but shipped as 1-byte fp8 {0,1} so it
feeds the PE directly.  The matmul runs in fp8 DoubleRow mode (256-deep
contraction per instruction): the mask is exact in fp8, and W' is scaled by
1.25 on the host so {0,0.1,1.0,1.1} map to exactly representable fp8 values
{0,0.125,1.25,1.375}; the host divides the result by 1.25.  sq is computed
on-chip from bf16 inputs and stored fp8 for the final elementwise reduce
(vector STT with accum), where its rounding noise averages out across 8.4M
terms.

Per-core HBM traffic: mask 8MB + W' 2MB + y/yh 2+2MB = 14MB (vs 25MB for
the f32 layout), all streamed behind the PE.  The mask is host-packed into
per-b-tile [128 ipart, 32 ichunk, 128 b] blocks so every DMA is 4KB
contiguous per partition.  Each core's inputs are rolled along b by
-core*16 rows so the SPMD-identical count reduction (columns 0:16 of each
mask tile) counts a disjoint set of mask rows on every core.

Each core emits per-partition partials of S = sum(sq*(m@W'))*1.25 and
cnt = sum(m); the host combines, divides by 1.25, and takes the sqrt.
"""

import numpy as np

B = 2048
N = 4096
NCORES = 8
NS = N // NCORES  # 512 columns per core
P = 128  # partitions
T = B // P  # 16 batch tiles
IC = N // P  # 32 i-chunks (contraction)
GP = IC // 2  # 16 DoubleRow i-pair chunks
EPS = 1e-6
WSCALE = 1.25  # makes {0, 0.1, 1.0, 1.1} exact in fp8e4m3

_CACHE: dict = {}


def build_program(repeat=1, nwarm=10):
    import concourse.bass as bass  # noqa: F401
    import concourse.tile as tile
    from concourse import bacc, mybir

    f32 = mybir.dt.float32
    bf16 = mybir.dt.bfloat16
    f8 = mybir.dt.float8e4
    Alu = mybir.AluOpType
    Act = mybir.ActivationFunctionType
    DR = mybir.MatmulPerfMode.DoubleRow

    nc = bacc.Bacc(
        "TRN2", target_bir_lowering=False, debug=False, num_devices=NCORES
    )

    yhat_d = nc.dram_tensor("yhat_s", [B, NS], bf16, kind="ExternalInput").ap()
    y_d = nc.dram_tensor("y_s", [B, NS], bf16, kind="ExternalInput").ap()
    mt_d = nc.dram_tensor("mt", [B, N], f8, kind="ExternalInput").ap()
    wps_d = nc.dram_tensor("wps", [N, NS], f8, kind="ExternalInput").ap()
    out_d = nc.dram_tensor("out", [P, 4], f32, kind="ExternalOutput").ap()

    yhat_v = yhat_d.rearrange("(t p) n -> p t n", p=P)  # [128, 16, 512]
    y_v = y_d.rearrange("(t p) n -> p t n", p=P)
    # host pre-packed: mt[t*128 + p, c*128 + j] = mask[t*128 + j, c*128 + p]
    mt_v = mt_d.rearrange("(t p) (c j) -> p t c j", p=P, c=IC)
    wps_v = wps_d.rearrange("(c p) n -> p c n", p=P)  # [128, 32, 512]

    with tile.TileContext(nc) as tc:
        with (
            tc.tile_pool(name="persist", bufs=1) as persist,
            tc.tile_pool(name="mtp", bufs=16) as mtp,
            tc.tile_pool(name="io", bufs=3) as iop,
            tc.tile_pool(name="scratch", bufs=2) as scratch,
            tc.tile_pool(name="psum", bufs=8, space="PSUM") as psum,
        ):
            for rep in range(repeat):
                sq_f8 = persist.tile(
                    [P, T, NS], f8, tag="sq", bufs=2, name=f"sq{rep}"
                )
                wps_sb = persist.tile(
                    [P, IC, NS], f8, tag="wps", bufs=2, name=f"wps{rep}"
                )
                acc = persist.tile([P, T], f32, tag="acc", bufs=2, name=f"acc{rep}")
                cntc = persist.tile([P, T], f32, tag="cnt", bufs=2, name=f"cnt{rep}")

                # ---- warm the PE clock gate during the DMA lead-in
                # (first rep only: later reps in the R-variant stay warm)
                for w_ in range(nwarm if rep == 0 else 0):
                    if w_ == 0:
                        dum = persist.tile(
                            [P, 2, NS], f8, tag="dum", bufs=1, name=f"dum{rep}"
                        )
                        nc.vector.memset(dum, 0.0)
                    ps_w = psum.tile(
                        [P, NS], f32, tag="ps", name=f"psw{rep}_{w_}"
                    )
                    nc.tensor.matmul(
                        ps_w,
                        lhsT=dum[:, :, 0:P],
                        rhs=dum,
                        start=True,
                        stop=True,
                        perf_mode=DR,
                    )

                # ---- DMA emission order is the schedule (sync HWDGE ring
                # is FIFO): W' quarters + first mask tiles up front, then
                # mask tiles interleaved with y/yhat pieces.
                mts = [None] * T
                yhp = [None] * (T // 2)
                yyp = [None] * (T // 2)

                def dma_wq(q):
                    nc.sync.dma_start(
                        out=wps_sb[:, 8 * q : 8 * (q + 1), :],
                        in_=wps_v[:, 8 * q : 8 * (q + 1), :],
                    )

                def dma_mt(t):
                    mts[t] = mtp.tile(
                        [P, IC, P], f8, tag="mt", name=f"mt{rep}_{t}"
                    )
                    nc.sync.dma_start(out=mts[t], in_=mt_v[:, t])

                def dma_piece(j):
                    yhp[j] = iop.tile(
                        [P, 2, NS], bf16, tag="yh", name=f"yh{rep}_{j}"
                    )
                    yyp[j] = iop.tile(
                        [P, 2, NS], bf16, tag="yy", name=f"yy{rep}_{j}"
                    )
                    sl = slice(2 * j, 2 * j + 2)
                    nc.sync.dma_start(out=yhp[j], in_=yhat_v[:, sl, :])
                    nc.sync.dma_start(out=yyp[j], in_=y_v[:, sl, :])

                dma_wq(0)
                dma_mt(0)
                dma_wq(1)
                dma_mt(1)
                dma_wq(2)
                dma_wq(3)
                dma_piece(0)
                dma_mt(2)
                dma_piece(1)
                dma_mt(3)
                dma_piece(2)
                dma_mt(4)
                dma_piece(3)
                dma_mt(5)
                dma_mt(6)
                dma_piece(4)
                dma_mt(7)
                dma_mt(8)
                dma_piece(5)
                dma_mt(9)
                dma_mt(10)
                dma_piece(6)
                dma_mt(11)
                dma_mt(12)
                dma_piece(7)
                dma_mt(13)
                dma_mt(14)
                dma_mt(15)

                # ---- main loop: per b-tile, 16 DoubleRow MMs (contraction
                # over all 4096 i), then one STT drain against sq.
                for t in range(T):
                    if t % 2 == 0:
                        j = t // 2
                        d_t = scratch.tile(
                            [P, 2, NS], bf16, tag="d", name=f"d{rep}_{j}"
                        )
                        nc.vector.tensor_sub(d_t, yyp[j], yhp[j])
                        nc.scalar.activation(
                            sq_f8[:, 2 * j : 2 * j + 2, :], d_t, Act.Square
                        )
                    ps_t = psum.tile([P, NS], f32, tag="ps", name=f"ps{rep}_{t}")
                    for g in range(GP):
                        nc.tensor.matmul(
                            ps_t,
                            lhsT=mts[t][:, 2 * g : 2 * g + 2, :],
                            rhs=wps_sb[:, 2 * g : 2 * g + 2, :],
                            start=(g == 0),
                            stop=(g == GP - 1),
                            perf_mode=DR,
                        )
                    tr = scratch.tile([P, NS], f32, tag="tr", name=f"tr{rep}_{t}")
                    nc.vector.scalar_tensor_tensor(
                        out=tr,
                        in0=ps_t,
                        scalar=1.0,
                        in1=sq_f8[:, t, :],
                        op0=Alu.mult,
                        op1=Alu.mult,
                        accum_out=acc[:, t : t + 1],
                    )
                    # disjoint-per-core count slice (inputs are b-rolled)
                    ct = scratch.tile(
                        [P, IC, 16], bf16, tag="ct", name=f"ct{rep}_{t}"
                    )
                    nc.scalar.activation(
                        ct,
                        mts[t][:, :, 0:16],
                        Act.Copy,
                        accum_out=cntc[:, t : t + 1],
                    )

                # pack partials: out[:, 0]=S*1.25, out[:, 2]=cnt
                out_sb = persist.tile([P, 4], f32, tag="os", bufs=2, name=f"os{rep}")
                nc.vector.memset(out_sb, 0.0)
                nc.vector.tensor_reduce(
                    out=out_sb[:, 0:1], in_=acc, axis=mybir.AxisListType.X,
                    op=Alu.add
                )
                nc.vector.tensor_reduce(
                    out=out_sb[:, 2:3], in_=cntc, axis=mybir.AxisListType.X,
                    op=Alu.add
                )
                nc.sync.dma_start(out=out_d, in_=out_sb)

    nc.compile()
    return nc


def make_in_maps(yhat, y, batch_mask, spots_neighbors):
    import ml_dtypes

    f8 = ml_dtypes.float8_e4m3
    bf16 = ml_dtypes.bfloat16

    mask_u8 = (np.ascontiguousarray(batch_mask) != 0).astype(np.uint8)
    yhat = np.ascontiguousarray(yhat, dtype=np.float32)
    y = np.ascontiguousarray(y, dtype=np.float32)
    w = np.ascontiguousarray(spots_neighbors, dtype=np.float32)
    idx = np.arange(NS)

    in_maps = []
    for c in range(NCORES):
        sl = slice(c * NS, (c + 1) * NS)
        roll = -c * (P // NCORES)  # distinct count rows per core
        # wps = 1.25 * (W[:, sl] + I[:, sl]) in fp8 (all values exact for
        # W entries in {0, 0.1})
        wps = w[:, sl] * WSCALE
        wps[c * NS + idx, idx] += WSCALE
        wps8 = wps.astype(f8)
        # b-rolled inputs
        mrow = np.roll(mask_u8, roll, axis=0)
        yh = np.roll(yhat[:, sl], roll, axis=0).astype(bf16)
        yy = np.roll(y[:, sl], roll, axis=0).astype(bf16)
        # mask -> fp8 {0,1} bytes, packed transposed per b-tile:
        # mt[t*128 + p, c*128 + j] = mask[t*128 + j, c*128 + p]
        m8 = (mrow * 0x38).astype(np.uint8)
        mt = (
            m8.reshape(T, P, IC, P)
            .transpose(0, 3, 2, 1)
            .reshape(B, N)
        )
        mt = np.ascontiguousarray(mt).view(f8)
        in_maps.append(
            {
                "yhat_s": np.ascontiguousarray(yh),
                "y_s": np.ascontiguousarray(yy),
                "mt": mt,
                "wps": np.ascontiguousarray(wps8),
            }
        )
    return in_maps


def combine_outs(outs):
    s = 0.0
    cnt = 0.0
    for o in outs:
        o64 = o.astype(np.float64)
        s += o64[:, 0].sum()
        cnt += o64[:, 2].sum()
    loss = np.sqrt(s / WSCALE / cnt + EPS)
    return np.array(loss, dtype=np.float32)


def kernel(yhat, y, batch_mask, spots_neighbors):
    from concourse.bass_utils import run_bass_kernel_spmd

    if "nc" not in _CACHE:
        _CACHE["nc"] = build_program()
    nc = _CACHE["nc"]
    in_maps = make_in_maps(yhat, y, batch_mask, spots_neighbors)
    res = run_bass_kernel_spmd(nc, in_maps, list(range(NCORES))).results
    return combine_outs([res[c]["out"] for c in range(NCORES)])
